# revision 1
# baseline (speedup 1.0000x reference)
"""DeepAR LSTM kernel for 8 Trainium2 NeuronCores.

Strategy (data-parallel over batch, 256 -> 8 cores x 32):
  * Transposed state layout: hT/cT are [K=128 partitions, B_loc=32 free].
  * Per step: 8 small matmuls accumulate all 4 gates into one PSUM tile
    [128, 128] (free = 4 gates x 32 batch, gate order i,f,o,g so one
    sigmoid instruction covers i,f,o and one tanh covers g).
  * Bias (b_ih + b_hh) is folded into the x-side matmul by augmenting x
    with a constant-1 row (contraction 65 instead of 64).
  * The full hidden history HT [128, L*32] stays resident in SBUF; the
    mu/logsigma heads run as a post-pass: HT chunks are the *stationary*
    matmul operand against [W_mu.T | W_sig.T] [128, 64].
  * Head biases are added on the host (free).
"""

import os
import sys
from contextlib import ExitStack

import numpy as np

sys.path.insert(0, "/opt/trn_rl_repo")

import concourse.bass as bass
import concourse.tile as tile
from concourse import bacc, mybir
from concourse.bass_utils import run_bass_kernel_spmd

L, B, IN, K, OBS = 1024, 256, 64, 128, 32
NCORES = 8
BL = B // NCORES  # 32 batch rows per core
TC = 128          # x-chunk length in steps (DMA double-buffered)

_LSTEPS = int(os.environ.get("KERNEL_LSTEPS", L))  # smoke-test override

F32 = mybir.dt.float32
AF = mybir.ActivationFunctionType

_cache = {}
RUN_KW = {}         # test harness may inject trace=True/tmpdir
LAST_RESULT = None  # BassKernelResults of the most recent run


def build_nc(nsteps: int) -> bass.Bass:
    nc = bacc.Bacc(
        "TRN2", target_bir_lowering=False, debug=False, num_devices=NCORES
    )
    ntc = min(TC, nsteps)
    xt = nc.dram_tensor("xt", [IN + 1, nsteps * BL], F32, kind="ExternalInput")
    whh = nc.dram_tensor("whh_t", [K, 4 * K], F32, kind="ExternalInput")
    wih = nc.dram_tensor("wih_t", [IN + 1, 4 * K], F32, kind="ExternalInput")
    whd = nc.dram_tensor("wheads", [K, 2 * OBS], F32, kind="ExternalInput")
    heads = nc.dram_tensor(
        "heads", [nsteps * BL, 2 * OBS], F32, kind="ExternalOutput"
    )

    with ExitStack() as ctx:
        tc = ctx.enter_context(tile.TileContext(nc))
        singles = ctx.enter_context(tc.tile_pool(name="singles", bufs=1))
        xpool = ctx.enter_context(tc.tile_pool(name="xchunk", bufs=2))
        sgp = ctx.enter_context(tc.tile_pool(name="sg", bufs=3))
        cp = ctx.enter_context(tc.tile_pool(name="c", bufs=2))
        thp = ctx.enter_context(tc.tile_pool(name="th", bufs=2))
        tmpp = ctx.enter_context(tc.tile_pool(name="tmp", bufs=2))
        psp = ctx.enter_context(tc.tile_pool(name="ps", bufs=4, space="PSUM"))
        hpsp = ctx.enter_context(tc.tile_pool(name="hps", bufs=2, space="PSUM"))
        dpsp = ctx.enter_context(tc.tile_pool(name="dps", bufs=1, space="PSUM"))
        outp = ctx.enter_context(tc.tile_pool(name="outt", bufs=3))

        whh_sb = singles.tile([K, 4 * K], F32)
        nc.sync.dma_start(whh_sb[:], whh[:])
        wih_sb = singles.tile([IN + 1, 4 * K], F32)
        nc.sync.dma_start(wih_sb[:], wih[:])
        whd_sb = singles.tile([K, 2 * OBS], F32)
        nc.sync.dma_start(whd_sb[:], whd[:])
        HT = singles.tile([K, nsteps * BL], F32)

        # A matmul's LDWEIGHTS can carry only ONE sync wait; make PE
        # observe each DMA semaphore via a throwaway 1x1 matmul so real
        # matmuls never need a DMA wait on top of a compute wait.
        dummy_ps = dpsp.tile([1, 1], F32)
        absorb_state = {"first": True}

        def pe_absorb(tile_ap):
            nc.tensor.matmul(
                dummy_ps[:], tile_ap[0:1, 0:1], tile_ap[0:1, 0:1],
                start=absorb_state["first"], stop=False,
                skip_group_check=True,
            )
            absorb_state["first"] = False

        pe_absorb(whh_sb)
        pe_absorb(wih_sb)
        pe_absorb(whd_sb)

        cprev = None
        xt_tile = None
        for t in range(nsteps):
            if t % ntc == 0:
                xt_tile = xpool.tile([IN + 1, ntc * BL], F32)
                nc.sync.dma_start(
                    xt_tile[:], xt[:, t * BL : (t + ntc) * BL]
                )
                pe_absorb(xt_tile)
            xs = xt_tile[:, (t % ntc) * BL : (t % ntc + 1) * BL]
            ps = psp.tile([K, 4 * BL], F32)
            # Open the PSUM slot with a DVE corner-write: it absorbs the
            # ACT slot-release + PE drain waits (DVE instructions may
            # carry several waits, matmuls only one), so the matmuls
            # below need at most a single DVE wait.
            nc.vector.tensor_copy(ps[0:1, 0:1], wih_sb[0:1, 0:1])
            # x-side matmul first, h-side second (waits on DVE h write).
            for g in range(4):
                dst = ps[:, g * BL : (g + 1) * BL]
                if t == 0:
                    nc.tensor.matmul(
                        dst, wih_sb[:, g * K : (g + 1) * K], xs,
                        start=True, stop=True,
                    )
                else:
                    hprev = HT[:, (t - 1) * BL : t * BL]
                    nc.tensor.matmul(
                        dst, wih_sb[:, g * K : (g + 1) * K], xs,
                        start=True, stop=False,
                    )
                    nc.tensor.matmul(
                        dst, whh_sb[:, g * K : (g + 1) * K], hprev,
                        start=False, stop=True,
                    )
            sg = sgp.tile([K, 4 * BL], F32)
            nc.scalar.activation(sg[:, 0 : 3 * BL], ps[:, 0 : 3 * BL], AF.Sigmoid)
            nc.scalar.activation(
                sg[:, 3 * BL : 4 * BL], ps[:, 3 * BL : 4 * BL], AF.Tanh
            )
            cnew = cp.tile([K, BL], F32)
            if t == 0:
                nc.vector.tensor_mul(
                    cnew[:], sg[:, 0:BL], sg[:, 3 * BL : 4 * BL]
                )
            else:
                fc = tmpp.tile([K, BL], F32)
                nc.vector.tensor_mul(fc[:], sg[:, BL : 2 * BL], cprev[:])
                ig = tmpp.tile([K, BL], F32)
                nc.vector.tensor_mul(ig[:], sg[:, 0:BL], sg[:, 3 * BL : 4 * BL])
                nc.vector.tensor_add(cnew[:], fc[:], ig[:])
            th = thp.tile([K, BL], F32)
            nc.scalar.activation(th[:], cnew[:], AF.Tanh)
            nc.vector.tensor_mul(
                HT[:, t * BL : (t + 1) * BL], sg[:, 2 * BL : 3 * BL], th[:]
            )
            cprev = cnew

        # mu / logsigma heads: HT chunks as stationary operand.
        nch = nsteps * BL // K
        for m in range(nch):
            hps = hpsp.tile([K, 2 * OBS], F32)
            nc.tensor.matmul(
                hps[:], HT[:, m * K : (m + 1) * K], whd_sb[:],
                start=True, stop=True,
            )
            ot = outp.tile([K, 2 * OBS], F32)
            if m % 2 == 0:
                nc.vector.tensor_copy(ot[:], hps[:])
            else:
                nc.scalar.copy(ot[:], hps[:])
            nc.sync.dma_start(heads[m * K : (m + 1) * K, :], ot[:])
    nc.compile()
    return nc


def _prep_weights(W_ih, W_hh, b_ih, b_hh, W_mu, W_sig):
    # torch gate order in rows: i(0:K) f(K:2K) g(2K:3K) o(3K:4K)
    # reorder rows to (i, f, o, g) so sigmoid covers a contiguous block
    perm = np.r_[0:K, K : 2 * K, 3 * K : 4 * K, 2 * K : 3 * K]
    whh_t = np.ascontiguousarray(W_hh[perm].T, np.float32)          # [K, 4K]
    bias = (b_ih + b_hh)[perm].astype(np.float32)
    wih_t = np.concatenate(
        [W_ih[perm].T, bias[None, :]], axis=0
    ).astype(np.float32)                                            # [IN+1, 4K]
    wheads = np.concatenate([W_mu.T, W_sig.T], axis=1).astype(np.float32)
    return whh_t, wih_t, wheads


def kernel(external_input_seq, W_ih, W_hh, b_ih, b_hh, W_mu, b_mu, W_sig, b_sig):
    nsteps = _LSTEPS
    x = np.asarray(external_input_seq, np.float32)[:nsteps]
    W_ih = np.asarray(W_ih, np.float32)
    W_hh = np.asarray(W_hh, np.float32)
    b_ih = np.asarray(b_ih, np.float32)
    b_hh = np.asarray(b_hh, np.float32)
    W_mu = np.asarray(W_mu, np.float32)
    b_mu = np.asarray(b_mu, np.float32)
    W_sig = np.asarray(W_sig, np.float32)
    b_sig = np.asarray(b_sig, np.float32)

    whh_t, wih_t, wheads = _prep_weights(W_ih, W_hh, b_ih, b_hh, W_mu, W_sig)

    if nsteps not in _cache:
        _cache[nsteps] = build_nc(nsteps)
    nc = _cache[nsteps]

    in_maps = []
    for c in range(NCORES):
        xc = x[:, c * BL : (c + 1) * BL, :]              # [nsteps, BL, IN]
        xt = np.empty((IN + 1, nsteps * BL), np.float32)
        xt[:IN] = xc.transpose(2, 0, 1).reshape(IN, nsteps * BL)
        xt[IN] = 1.0
        in_maps.append(
            {"xt": xt, "whh_t": whh_t, "wih_t": wih_t, "wheads": wheads}
        )

    res = run_bass_kernel_spmd(
        nc, in_maps, core_ids=list(range(NCORES)), **RUN_KW
    )
    global LAST_RESULT
    LAST_RESULT = res

    mu = np.empty((nsteps, B, OBS), np.float32)
    sig = np.empty((nsteps, B, OBS), np.float32)
    for c in range(NCORES):
        h = res.results[c]["heads"].reshape(nsteps, BL, 2 * OBS)
        mu[:, c * BL : (c + 1) * BL, :] = h[:, :, :OBS]
        sig[:, c * BL : (c + 1) * BL, :] = h[:, :, OBS:]
    mu += b_mu
    sig += b_sig
    return mu, sig



# revision 4
# speedup vs baseline: 2.4064x; 2.4064x over previous
"""DeepAR LSTM kernel for 8 Trainium2 NeuronCores.

Strategy (data-parallel over batch, 256 -> 8 cores x 32):
  * Transposed state layout: h/c are [K=128 partitions, B_loc=32 free].
  * bf16 matmul inputs everywhere (fp32 PSUM accumulation): single-pass
    matmuls + fast weight load, vs fp32's HIGH/LOW double-pass.
  * x-side gate contributions (Wx@x + bias) are precomputed per 4-step
    block directly INTO the PSUM bank (start=True), off the critical
    path; the per-step recurrent matmuls accumulate on top (start=False).
    Bias is folded in via a constant-1 row on x (contraction 65).
  * Gate order i,f,o,g with the g-gate pre-activation scaled by 2 on the
    host so ONE sigmoid instruction covers all four gates:
    tanh(z) = 2*sigmoid(2z) - 1, fixed up in the DVE cell update.
  * mu/logsigma head matmuls run interleaved (one per 4-step block, one
    block behind) on otherwise-idle PE cycles; results stream out via
    DMA during the recurrence. Head biases are added on the host.
"""

import os
import sys
from contextlib import ExitStack

import ml_dtypes
import numpy as np

sys.path.insert(0, "/opt/trn_rl_repo")

import concourse.bass as bass
import concourse.tile as tile
from concourse import bacc, mybir
from concourse.bass_utils import run_bass_kernel_spmd

L, B, IN, K, OBS = 1024, 256, 64, 128, 32
NCORES = 8
BL = B // NCORES  # 32 batch rows per core
CH = 128          # x-chunk length in steps (DMA double-buffered)
BLK = 4           # steps per PSUM gate block (4*128 fp32 = 1 bank)

_LSTEPS = int(os.environ.get("KERNEL_LSTEPS", L))  # smoke-test override

F32 = mybir.dt.float32
BF16 = mybir.dt.bfloat16
AF = mybir.ActivationFunctionType
ALU = mybir.AluOpType

_cache = {}
RUN_KW = {}         # test harness may inject trace=True/tmpdir
LAST_RESULT = None  # BassKernelResults of the most recent run


def build_nc(nsteps: int) -> bass.Bass:
    nc = bacc.Bacc(
        "TRN2", target_bir_lowering=False, debug=False, num_devices=NCORES
    )
    nch = min(CH, nsteps)
    nblk = nsteps // BLK
    xt = nc.dram_tensor("xt", [IN + 1, nsteps * BL], BF16, kind="ExternalInput")
    whh = nc.dram_tensor("whh_t", [K, 4 * K], BF16, kind="ExternalInput")
    wih = nc.dram_tensor("wih_t", [IN + 1, 4 * K], BF16, kind="ExternalInput")
    whd = nc.dram_tensor("wheads", [K, 2 * OBS], BF16, kind="ExternalInput")
    heads = nc.dram_tensor(
        "heads", [nsteps * BL, 2 * OBS], F32, kind="ExternalOutput"
    )

    with ExitStack() as ctx:
        tc = ctx.enter_context(tile.TileContext(nc))
        singles = ctx.enter_context(tc.tile_pool(name="singles", bufs=1))
        xpool = ctx.enter_context(tc.tile_pool(name="xchunk", bufs=2))
        sgp = ctx.enter_context(tc.tile_pool(name="sg", bufs=2))
        cp = ctx.enter_context(tc.tile_pool(name="c", bufs=2))
        thp = ctx.enter_context(tc.tile_pool(name="th", bufs=2))
        tmpp = ctx.enter_context(tc.tile_pool(name="tmp", bufs=2))
        gpsp = ctx.enter_context(tc.tile_pool(name="gps", bufs=3, space="PSUM"))
        hpsp = ctx.enter_context(tc.tile_pool(name="hps", bufs=2, space="PSUM"))
        dpsp = ctx.enter_context(tc.tile_pool(name="dps", bufs=1, space="PSUM"))
        outp = ctx.enter_context(tc.tile_pool(name="outt", bufs=3))

        whh_sb = singles.tile([K, 4 * K], BF16)
        nc.sync.dma_start(whh_sb[:], whh[:])
        wih_sb = singles.tile([IN + 1, 4 * K], BF16)
        nc.sync.dma_start(wih_sb[:], wih[:])
        whd_sb = singles.tile([K, 2 * OBS], BF16)
        nc.sync.dma_start(whd_sb[:], whd[:])
        HT = singles.tile([K, nsteps * BL], BF16)

        # A matmul's LDWEIGHTS can carry only ONE sync wait; make PE
        # observe each DMA semaphore via a throwaway 1x1 matmul so real
        # matmuls never need a DMA wait on top of a compute wait.
        dummy_ps = dpsp.tile([1, 1], F32)
        absorb_state = {"first": True}

        def pe_absorb(tile_ap):
            nc.tensor.matmul(
                dummy_ps[:], tile_ap[0:1, 0:1], tile_ap[0:1, 0:1],
                start=absorb_state["first"], stop=False,
                skip_group_check=True,
            )
            absorb_state["first"] = False

        pe_absorb(whh_sb)
        pe_absorb(wih_sb)
        pe_absorb(whd_sb)

        chunk_tiles = {}

        def load_chunk(c):
            xt_tile = xpool.tile([IN + 1, nch * BL], BF16, name=f"xt_c{c % 2}")
            nc.sync.dma_start(
                xt_tile[:], xt[:, c * nch * BL : (c + 1) * nch * BL]
            )
            pe_absorb(xt_tile)
            chunk_tiles[c] = xt_tile

        def issue_xmm(b):
            # Precompute x-side gate pre-activations for block b straight
            # into a fresh PSUM bank (start=True clears it).
            t0 = b * BLK
            c = t0 // nch
            xt_tile = chunk_tiles[c]
            col0 = (t0 - c * nch) * BL
            ps = gpsp.tile([K, BLK, 4 * BL], F32, name="gateps")
            for g in range(4):
                # start=True pending-zeroes the whole 2KB bank, so only
                # the first matmul of the block may carry it.
                nc.tensor.matmul(
                    ps[:, :, g * BL : (g + 1) * BL],
                    wih_sb[:, g * K : (g + 1) * K],
                    xt_tile[:, col0 : col0 + BLK * BL],
                    start=(g == 0), stop=False, skip_group_check=True,
                )
            return ps

        def issue_heads(b):
            # mu/logsigma heads for (finished) block b: HT chunk is the
            # stationary operand against [W_mu.T | W_sig.T].
            hps = hpsp.tile([K, 2 * OBS], F32, name="headps")
            nc.tensor.matmul(
                hps[:], HT[:, b * K : (b + 1) * K], whd_sb[:],
                start=True, stop=True, skip_group_check=True,
            )
            return hps

        load_chunk(0)
        ps_cur = issue_xmm(0)
        ps_next = issue_xmm(1) if nblk > 1 else None

        cprev = None
        pend_head = None  # (psum_tile, block) awaiting copy+DMA
        for t in range(nsteps):
            s = t % BLK
            if t % BLK == 0 and t > 0:
                ps_cur, ps_next = ps_next, None
            if t % nch == 0 and (t + nch) < nsteps:
                load_chunk(t // nch + 1)

            if t > 0:
                hprev = HT[:, (t - 1) * BL : t * BL]
                for g in range(4):
                    nc.tensor.matmul(
                        ps_cur[:, s, g * BL : (g + 1) * BL],
                        whh_sb[:, g * K : (g + 1) * K],
                        hprev,
                        start=False, stop=(g == 3 and s == BLK - 1),
                        skip_group_check=True,
                    )
            if s == 0 and t > 0:
                b = t // BLK
                if b + 1 < nblk:
                    ps_next = issue_xmm(b + 1)
                pend_head = (issue_heads(b - 1), b - 1)

            # One sigmoid covers i,f,o and the 2x-prescaled g.
            sg = sgp.tile([K, 4 * BL], F32)
            nc.scalar.activation(sg[:], ps_cur[:, s, :], AF.Sigmoid)

            # c = f*c_prev + i*(2*sg_g - 1)
            A = tmpp.tile([K, BL], F32, name="A")
            nc.vector.tensor_mul(A[:], sg[:, 0:BL], sg[:, 3 * BL : 4 * BL])
            if t == 0:
                cnew = cp.tile([K, BL], F32, name="cnew")
                nc.vector.scalar_tensor_tensor(
                    cnew[:], A[:], 2.0, sg[:, 0:BL], ALU.mult, ALU.subtract
                )
            else:
                m1 = tmpp.tile([K, BL], F32, name="m1")
                nc.vector.tensor_mul(m1[:], sg[:, BL : 2 * BL], cprev[:])
                Bt = tmpp.tile([K, BL], F32, name="Bt")
                nc.vector.scalar_tensor_tensor(
                    Bt[:], A[:], 2.0, sg[:, 0:BL], ALU.mult, ALU.subtract
                )
                cnew = cp.tile([K, BL], F32, name="cnew")
                nc.vector.tensor_add(cnew[:], m1[:], Bt[:])
            th = thp.tile([K, BL], F32)
            nc.scalar.activation(th[:], cnew[:], AF.Tanh)
            nc.vector.tensor_mul(
                HT[:, t * BL : (t + 1) * BL], sg[:, 2 * BL : 3 * BL], th[:]
            )
            cprev = cnew

            if pend_head is not None:
                hps, hb = pend_head
                pend_head = None
                ot = outp.tile([K, 2 * OBS], F32)
                nc.vector.tensor_copy(ot[:], hps[:])
                nc.sync.dma_start(heads[hb * K : (hb + 1) * K, :], ot[:])

        # last block's heads
        hps = issue_heads(nblk - 1)
        ot = outp.tile([K, 2 * OBS], F32, name="ot_last")
        nc.vector.tensor_copy(ot[:], hps[:])
        nc.sync.dma_start(heads[(nblk - 1) * K : nblk * K, :], ot[:])
    nc.compile()
    return nc


def _prep_weights(W_ih, W_hh, b_ih, b_hh, W_mu, W_sig):
    # torch gate order in rows: i(0:K) f(K:2K) g(2K:3K) o(3K:4K)
    # reorder rows to (i, f, o, g) so one sigmoid covers everything;
    # the g block is pre-scaled by 2 for tanh(z) = 2*sigmoid(2z) - 1.
    perm = np.r_[0:K, K : 2 * K, 3 * K : 4 * K, 2 * K : 3 * K]
    whh_t = np.ascontiguousarray(W_hh[perm].T, np.float32)          # [K, 4K]
    bias = (b_ih + b_hh)[perm].astype(np.float32)
    wih_t = np.concatenate(
        [W_ih[perm].T, bias[None, :]], axis=0
    ).astype(np.float32)                                            # [IN+1, 4K]
    whh_t[:, 3 * K :] *= 2.0
    wih_t[:, 3 * K :] *= 2.0
    wheads = np.concatenate([W_mu.T, W_sig.T], axis=1).astype(np.float32)
    bf = ml_dtypes.bfloat16
    return whh_t.astype(bf), wih_t.astype(bf), wheads.astype(bf)


def kernel(external_input_seq, W_ih, W_hh, b_ih, b_hh, W_mu, b_mu, W_sig, b_sig):
    nsteps = _LSTEPS
    x = np.asarray(external_input_seq, np.float32)[:nsteps]
    W_ih = np.asarray(W_ih, np.float32)
    W_hh = np.asarray(W_hh, np.float32)
    b_ih = np.asarray(b_ih, np.float32)
    b_hh = np.asarray(b_hh, np.float32)
    W_mu = np.asarray(W_mu, np.float32)
    b_mu = np.asarray(b_mu, np.float32)
    W_sig = np.asarray(W_sig, np.float32)
    b_sig = np.asarray(b_sig, np.float32)

    whh_t, wih_t, wheads = _prep_weights(W_ih, W_hh, b_ih, b_hh, W_mu, W_sig)

    if nsteps not in _cache:
        _cache[nsteps] = build_nc(nsteps)
    nc = _cache[nsteps]

    bf = ml_dtypes.bfloat16
    in_maps = []
    for c in range(NCORES):
        xc = x[:, c * BL : (c + 1) * BL, :]              # [nsteps, BL, IN]
        xt = np.empty((IN + 1, nsteps * BL), np.float32)
        xt[:IN] = xc.transpose(2, 0, 1).reshape(IN, nsteps * BL)
        xt[IN] = 1.0
        in_maps.append(
            {"xt": xt.astype(bf), "whh_t": whh_t, "wih_t": wih_t,
             "wheads": wheads}
        )

    res = run_bass_kernel_spmd(
        nc, in_maps, core_ids=list(range(NCORES)), **RUN_KW
    )
    global LAST_RESULT
    LAST_RESULT = res

    mu = np.empty((nsteps, B, OBS), np.float32)
    sig = np.empty((nsteps, B, OBS), np.float32)
    for c in range(NCORES):
        h = res.results[c]["heads"].reshape(nsteps, BL, 2 * OBS)
        mu[:, c * BL : (c + 1) * BL, :] = h[:, :, :OBS]
        sig[:, c * BL : (c + 1) * BL, :] = h[:, :, OBS:]
    mu += b_mu
    sig += b_sig
    return mu, sig


# revision 6
# speedup vs baseline: 2.4549x; 1.0202x over previous
"""DeepAR LSTM kernel for 8 Trainium2 NeuronCores.

Strategy (data-parallel over batch, 256 -> 8 cores x 32):
  * Transposed state layout: h/c are [K=128 partitions, B_loc=32 free].
  * bf16 matmul inputs everywhere (fp32 PSUM accumulation): single-pass
    matmuls + fast weight load, vs fp32's HIGH/LOW double-pass.
  * x-side gate contributions (Wx@x + bias) are precomputed per 4-step
    block directly INTO the PSUM bank (start=True), off the critical
    path; the per-step recurrent matmuls accumulate on top (start=False).
    Bias is folded in via a constant-1 row on x (contraction 65).
  * Gate order i,f,o,g with the g-gate pre-activation scaled by 2 on the
    host so ONE sigmoid instruction covers all four gates:
    tanh(z) = 2*sigmoid(2z) - 1, fixed up in the DVE cell update.
  * mu/logsigma head matmuls run interleaved (one per 4-step block, one
    block behind) on otherwise-idle PE cycles; results stream out via
    DMA during the recurrence. Head biases are added on the host.
"""

import os
import sys
from contextlib import ExitStack

import ml_dtypes
import numpy as np

sys.path.insert(0, "/opt/trn_rl_repo")

import concourse.bass as bass
import concourse.tile as tile
from concourse import bacc, mybir
from concourse.bass_utils import run_bass_kernel_spmd

L, B, IN, K, OBS = 1024, 256, 64, 128, 32
NCORES = 8
BL = B // NCORES  # 32 batch rows per core
CH = 128          # x-chunk length in steps (DMA double-buffered)
BLK = 4           # steps per PSUM gate block (4*128 fp32 = 1 bank)

_LSTEPS = int(os.environ.get("KERNEL_LSTEPS", L))  # smoke-test override

F32 = mybir.dt.float32
BF16 = mybir.dt.bfloat16
AF = mybir.ActivationFunctionType
ALU = mybir.AluOpType

_cache = {}
RUN_KW = {}         # test harness may inject trace=True/tmpdir
LAST_RESULT = None  # BassKernelResults of the most recent run


def build_nc(nsteps: int) -> bass.Bass:
    nc = bacc.Bacc(
        "TRN2", target_bir_lowering=False, debug=False, num_devices=NCORES
    )
    nch = min(CH, nsteps)
    nblk = nsteps // BLK
    xt = nc.dram_tensor("xt", [IN + 1, nsteps * BL], BF16, kind="ExternalInput")
    whh = nc.dram_tensor("whh_t", [K, 4 * K], BF16, kind="ExternalInput")
    wih = nc.dram_tensor("wih_t", [IN + 1, 4 * K], BF16, kind="ExternalInput")
    whd = nc.dram_tensor("wheads", [K, 2 * OBS], BF16, kind="ExternalInput")
    heads = nc.dram_tensor(
        "heads", [nsteps * BL, 2 * OBS], F32, kind="ExternalOutput"
    )

    with ExitStack() as ctx:
        tc = ctx.enter_context(tile.TileContext(nc))
        singles = ctx.enter_context(tc.tile_pool(name="singles", bufs=1))
        xpool = ctx.enter_context(tc.tile_pool(name="xchunk", bufs=2))
        sgp = ctx.enter_context(tc.tile_pool(name="sg", bufs=2))
        cp = ctx.enter_context(tc.tile_pool(name="c", bufs=2))
        thp = ctx.enter_context(tc.tile_pool(name="th", bufs=2))
        tmpp = ctx.enter_context(tc.tile_pool(name="tmp", bufs=2))
        gpsp = ctx.enter_context(tc.tile_pool(name="gps", bufs=3, space="PSUM"))
        hpsp = ctx.enter_context(tc.tile_pool(name="hps", bufs=2, space="PSUM"))
        dpsp = ctx.enter_context(tc.tile_pool(name="dps", bufs=1, space="PSUM"))
        outp = ctx.enter_context(tc.tile_pool(name="outt", bufs=3))

        whh_sb = singles.tile([K, 4 * K], BF16)
        nc.sync.dma_start(whh_sb[:], whh[:])
        wih_sb = singles.tile([IN + 1, 4 * K], BF16)
        nc.sync.dma_start(wih_sb[:], wih[:])
        whd_sb = singles.tile([K, 2 * OBS], BF16)
        nc.sync.dma_start(whd_sb[:], whd[:])
        HT = singles.tile([K, nsteps * BL], BF16)

        # A matmul's LDWEIGHTS can carry only ONE sync wait; make PE
        # observe each DMA semaphore via a throwaway 1x1 matmul so real
        # matmuls never need a DMA wait on top of a compute wait.
        dummy_ps = dpsp.tile([1, 1], F32)
        absorb_state = {"first": True}

        def pe_absorb(tile_ap):
            nc.tensor.matmul(
                dummy_ps[:], tile_ap[0:1, 0:1], tile_ap[0:1, 0:1],
                start=absorb_state["first"], stop=False,
                skip_group_check=True,
            )
            absorb_state["first"] = False

        pe_absorb(whh_sb)
        pe_absorb(wih_sb)
        pe_absorb(whd_sb)

        chunk_tiles = {}

        def load_chunk(c):
            xt_tile = xpool.tile([IN + 1, nch * BL], BF16, name=f"xt_c{c % 2}")
            nc.sync.dma_start(
                xt_tile[:], xt[:, c * nch * BL : (c + 1) * nch * BL]
            )
            pe_absorb(xt_tile)
            chunk_tiles[c] = xt_tile

        def issue_xmm(b):
            # Precompute x-side gate pre-activations for block b straight
            # into a fresh PSUM bank (start=True clears it).
            t0 = b * BLK
            c = t0 // nch
            xt_tile = chunk_tiles[c]
            col0 = (t0 - c * nch) * BL
            ps = gpsp.tile([K, BLK, 4 * BL], F32, name="gateps")
            for g in range(4):
                # start=True pending-zeroes the whole 2KB bank, so only
                # the first matmul of the block may carry it.
                nc.tensor.matmul(
                    ps[:, :, g * BL : (g + 1) * BL],
                    wih_sb[:, g * K : (g + 1) * K],
                    xt_tile[:, col0 : col0 + BLK * BL],
                    start=(g == 0), stop=False, skip_group_check=True,
                )
            return ps

        def issue_heads(b):
            # mu/logsigma heads for (finished) block b: HT chunk is the
            # stationary operand against [W_mu.T | W_sig.T].
            hps = hpsp.tile([K, 2 * OBS], F32, name="headps")
            nc.tensor.matmul(
                hps[:], HT[:, b * K : (b + 1) * K], whd_sb[:],
                start=True, stop=True, skip_group_check=True,
            )
            return hps

        load_chunk(0)
        ps_cur = issue_xmm(0)
        ps_next = issue_xmm(1) if nblk > 1 else None

        cprev = None
        pend_head = None  # (psum_tile, block) awaiting copy+DMA
        for t in range(nsteps):
            s = t % BLK
            if t % BLK == 0 and t > 0:
                ps_cur, ps_next = ps_next, None
            if t % nch == 0 and (t + nch) < nsteps:
                load_chunk(t // nch + 1)

            if t > 0:
                hprev = HT[:, (t - 1) * BL : t * BL]
                for g in range(4):
                    nc.tensor.matmul(
                        ps_cur[:, s, g * BL : (g + 1) * BL],
                        whh_sb[:, g * K : (g + 1) * K],
                        hprev,
                        start=False, stop=(g == 3 and s == BLK - 1),
                        skip_group_check=True,
                    )
            if s == 0 and t > 0:
                b = t // BLK
                if b + 1 < nblk:
                    ps_next = issue_xmm(b + 1)
                pend_head = (issue_heads(b - 1), b - 1)

            # Gate order i,f,g,o: one sigmoid covers i,f and the
            # 2x-prescaled g as soon as the g-matmul lands; the o-matmul
            # and sigma(o) run off the critical path.
            sg = sgp.tile([K, 4 * BL], F32)
            nc.scalar.activation(sg[:, 0 : 3 * BL], ps_cur[:, s, 0 : 3 * BL],
                                 AF.Sigmoid)

            # c = f*c_prev + i*(2*sg_g - 1)
            A = tmpp.tile([K, BL], F32, name="A")
            nc.vector.tensor_mul(A[:], sg[:, 0:BL], sg[:, 2 * BL : 3 * BL])
            if t == 0:
                cnew = cp.tile([K, BL], F32, name="cnew")
                nc.vector.scalar_tensor_tensor(
                    cnew[:], A[:], 2.0, sg[:, 0:BL], ALU.mult, ALU.subtract
                )
            else:
                m1 = tmpp.tile([K, BL], F32, name="m1")
                nc.vector.tensor_mul(m1[:], sg[:, BL : 2 * BL], cprev[:])
                Bt = tmpp.tile([K, BL], F32, name="Bt")
                nc.vector.scalar_tensor_tensor(
                    Bt[:], A[:], 2.0, sg[:, 0:BL], ALU.mult, ALU.subtract
                )
                cnew = cp.tile([K, BL], F32, name="cnew")
                nc.vector.tensor_add(cnew[:], m1[:], Bt[:])
            nc.scalar.activation(sg[:, 3 * BL : 4 * BL],
                                 ps_cur[:, s, 3 * BL : 4 * BL], AF.Sigmoid)
            th = thp.tile([K, BL], F32)
            nc.scalar.activation(th[:], cnew[:], AF.Tanh)
            nc.vector.tensor_mul(
                HT[:, t * BL : (t + 1) * BL], sg[:, 3 * BL : 4 * BL], th[:]
            )
            cprev = cnew

            if pend_head is not None:
                hps, hb = pend_head
                pend_head = None
                ot = outp.tile([K, 2 * OBS], F32)
                nc.vector.tensor_copy(ot[:], hps[:])
                nc.sync.dma_start(heads[hb * K : (hb + 1) * K, :], ot[:])

        # last block's heads
        hps = issue_heads(nblk - 1)
        ot = outp.tile([K, 2 * OBS], F32, name="ot_last")
        nc.vector.tensor_copy(ot[:], hps[:])
        nc.sync.dma_start(heads[(nblk - 1) * K : nblk * K, :], ot[:])
    nc.compile()
    return nc


def _prep_weights(W_ih, W_hh, b_ih, b_hh, W_mu, W_sig):
    # torch gate order i(0:K) f(K:2K) g(2K:3K) o(3K:4K) is kept as-is;
    # the g block is pre-scaled by 2 for tanh(z) = 2*sigmoid(2z) - 1.
    whh_t = np.ascontiguousarray(W_hh.T, np.float32)                # [K, 4K]
    bias = (b_ih + b_hh).astype(np.float32)
    wih_t = np.concatenate(
        [W_ih.T, bias[None, :]], axis=0
    ).astype(np.float32)                                            # [IN+1, 4K]
    whh_t[:, 2 * K : 3 * K] *= 2.0
    wih_t[:, 2 * K : 3 * K] *= 2.0
    wheads = np.concatenate([W_mu.T, W_sig.T], axis=1).astype(np.float32)
    bf = ml_dtypes.bfloat16
    return whh_t.astype(bf), wih_t.astype(bf), wheads.astype(bf)


def kernel(external_input_seq, W_ih, W_hh, b_ih, b_hh, W_mu, b_mu, W_sig, b_sig):
    nsteps = _LSTEPS
    x = np.asarray(external_input_seq, np.float32)[:nsteps]
    W_ih = np.asarray(W_ih, np.float32)
    W_hh = np.asarray(W_hh, np.float32)
    b_ih = np.asarray(b_ih, np.float32)
    b_hh = np.asarray(b_hh, np.float32)
    W_mu = np.asarray(W_mu, np.float32)
    b_mu = np.asarray(b_mu, np.float32)
    W_sig = np.asarray(W_sig, np.float32)
    b_sig = np.asarray(b_sig, np.float32)

    whh_t, wih_t, wheads = _prep_weights(W_ih, W_hh, b_ih, b_hh, W_mu, W_sig)

    if nsteps not in _cache:
        _cache[nsteps] = build_nc(nsteps)
    nc = _cache[nsteps]

    bf = ml_dtypes.bfloat16
    in_maps = []
    for c in range(NCORES):
        xc = x[:, c * BL : (c + 1) * BL, :]              # [nsteps, BL, IN]
        xt = np.empty((IN + 1, nsteps * BL), np.float32)
        xt[:IN] = xc.transpose(2, 0, 1).reshape(IN, nsteps * BL)
        xt[IN] = 1.0
        in_maps.append(
            {"xt": xt.astype(bf), "whh_t": whh_t, "wih_t": wih_t,
             "wheads": wheads}
        )

    res = run_bass_kernel_spmd(
        nc, in_maps, core_ids=list(range(NCORES)), **RUN_KW
    )
    global LAST_RESULT
    LAST_RESULT = res

    mu = np.empty((nsteps, B, OBS), np.float32)
    sig = np.empty((nsteps, B, OBS), np.float32)
    for c in range(NCORES):
        h = res.results[c]["heads"].reshape(nsteps, BL, 2 * OBS)
        mu[:, c * BL : (c + 1) * BL, :] = h[:, :, :OBS]
        sig[:, c * BL : (c + 1) * BL, :] = h[:, :, OBS:]
    mu += b_mu
    sig += b_sig
    return mu, sig


# revision 9
# speedup vs baseline: 2.4771x; 1.0091x over previous
"""DeepAR LSTM kernel for 8 Trainium2 NeuronCores.

Strategy (data-parallel over batch, 256 -> 8 cores x 32):
  * Transposed state layout: h/c are [K=128 partitions, B_loc=32 free].
  * bf16 matmul inputs everywhere (fp32 PSUM accumulation): single-pass
    matmuls + fast weight load, vs fp32's HIGH/LOW double-pass.
  * x-side gate contributions (Wx@x + bias) are precomputed per 4-step
    block directly INTO the PSUM bank (start=True), off the critical
    path; the per-step recurrent matmuls accumulate on top (start=False).
    Bias is folded in via a constant-1 row on x (contraction 65).
  * Gate order i,f,o,g with the g-gate pre-activation scaled by 2 on the
    host so ONE sigmoid instruction covers all four gates:
    tanh(z) = 2*sigmoid(2z) - 1, fixed up in the DVE cell update.
  * mu/logsigma head matmuls run interleaved (one per 4-step block, one
    block behind) on otherwise-idle PE cycles; results stream out via
    DMA during the recurrence. Head biases are added on the host.
"""

import os
import sys
from contextlib import ExitStack

import ml_dtypes
import numpy as np

sys.path.insert(0, "/opt/trn_rl_repo")

import concourse.bass as bass
import concourse.tile as tile
from concourse import bacc, mybir
from concourse.bass_utils import run_bass_kernel_spmd

L, B, IN, K, OBS = 1024, 256, 64, 128, 32
NCORES = 8
BL = B // NCORES  # 32 batch rows per core
CH = 128          # x-chunk length in steps (DMA double-buffered)
BLK = 4           # steps per PSUM gate block (4*128 fp32 = 1 bank)

_LSTEPS = int(os.environ.get("KERNEL_LSTEPS", L))  # smoke-test override

F32 = mybir.dt.float32
BF16 = mybir.dt.bfloat16
AF = mybir.ActivationFunctionType
ALU = mybir.AluOpType

_cache = {}
RUN_KW = {}         # test harness may inject trace=True/tmpdir
LAST_RESULT = None  # BassKernelResults of the most recent run


def build_nc(nsteps: int) -> bass.Bass:
    nc = bacc.Bacc(
        "TRN2", target_bir_lowering=False, debug=False, num_devices=NCORES
    )
    nch = min(CH, nsteps)
    nblk = nsteps // BLK
    xt = nc.dram_tensor("xt", [IN + 1, nsteps * BL], BF16, kind="ExternalInput")
    whh = nc.dram_tensor("whh_t", [K, 4 * K], BF16, kind="ExternalInput")
    wih = nc.dram_tensor("wih_t", [IN + 1, 4 * K], BF16, kind="ExternalInput")
    whd = nc.dram_tensor("wheads", [K, 2 * OBS], BF16, kind="ExternalInput")
    heads = nc.dram_tensor(
        "heads", [nsteps * BL, 2 * OBS], F32, kind="ExternalOutput"
    )

    with ExitStack() as ctx:
        tc = ctx.enter_context(tile.TileContext(nc))
        singles = ctx.enter_context(tc.tile_pool(name="singles", bufs=1))
        xpool = ctx.enter_context(tc.tile_pool(name="xchunk", bufs=2))
        sgp = ctx.enter_context(tc.tile_pool(name="sg", bufs=4))
        cp = ctx.enter_context(tc.tile_pool(name="c", bufs=2))
        thp = ctx.enter_context(tc.tile_pool(name="th", bufs=2))
        tmpp = ctx.enter_context(tc.tile_pool(name="tmp", bufs=2))
        gpsp = ctx.enter_context(tc.tile_pool(name="gps", bufs=3, space="PSUM"))
        hpsp = ctx.enter_context(tc.tile_pool(name="hps", bufs=2, space="PSUM"))
        dpsp = ctx.enter_context(tc.tile_pool(name="dps", bufs=1, space="PSUM"))
        outp = ctx.enter_context(tc.tile_pool(name="outt", bufs=3))

        whh_sb = singles.tile([K, 4 * K], BF16)
        nc.sync.dma_start(whh_sb[:], whh[:])
        wih_sb = singles.tile([IN + 1, 4 * K], BF16)
        nc.sync.dma_start(wih_sb[:], wih[:])
        whd_sb = singles.tile([K, 2 * OBS], BF16)
        nc.sync.dma_start(whd_sb[:], whd[:])
        HT = singles.tile([K, nsteps * BL], BF16)

        # A matmul's LDWEIGHTS can carry only ONE sync wait; make PE
        # observe each DMA semaphore via a throwaway 1x1 matmul so real
        # matmuls never need a DMA wait on top of a compute wait.
        dummy_ps = dpsp.tile([1, 1], F32)
        absorb_state = {"first": True}

        def pe_absorb(tile_ap):
            nc.tensor.matmul(
                dummy_ps[:], tile_ap[0:1, 0:1], tile_ap[0:1, 0:1],
                start=absorb_state["first"], stop=False,
                skip_group_check=True,
            )
            absorb_state["first"] = False

        pe_absorb(whh_sb)
        pe_absorb(wih_sb)
        pe_absorb(whd_sb)

        chunk_tiles = {}

        def load_chunk(c):
            xt_tile = xpool.tile([IN + 1, nch * BL], BF16, name=f"xt_c{c % 2}")
            nc.sync.dma_start(
                xt_tile[:], xt[:, c * nch * BL : (c + 1) * nch * BL]
            )
            pe_absorb(xt_tile)
            chunk_tiles[c] = xt_tile

        def issue_xmm(b):
            # Precompute x-side gate pre-activations for block b straight
            # into a fresh PSUM bank (start=True clears it).
            t0 = b * BLK
            c = t0 // nch
            xt_tile = chunk_tiles[c]
            col0 = (t0 - c * nch) * BL
            ps = gpsp.tile([K, BLK, 4 * BL], F32, name="gateps")
            for g in range(4):
                # start=True pending-zeroes the whole 2KB bank, so only
                # the first matmul of the block may carry it.
                nc.tensor.matmul(
                    ps[:, :, g * BL : (g + 1) * BL],
                    wih_sb[:, g * K : (g + 1) * K],
                    xt_tile[:, col0 : col0 + BLK * BL],
                    start=(g == 0), stop=False, skip_group_check=True,
                )
            return ps

        def issue_heads(b):
            # mu/logsigma heads for (finished) block b: HT chunk is the
            # stationary operand against [W_mu.T | W_sig.T].
            hps = hpsp.tile([K, 2 * OBS], F32, name="headps")
            nc.tensor.matmul(
                hps[:], HT[:, b * K : (b + 1) * K], whd_sb[:],
                start=True, stop=True, skip_group_check=True,
            )
            return hps

        load_chunk(0)
        ps_cur = issue_xmm(0)
        ps_next = issue_xmm(1) if nblk > 1 else None

        cprev = None
        pend_head = None  # (psum_tile, block) awaiting copy+DMA
        for t in range(nsteps):
            s = t % BLK
            if t % BLK == 0 and t > 0:
                ps_cur, ps_next = ps_next, None
            if t % nch == 0 and (t + nch) < nsteps:
                load_chunk(t // nch + 1)

            if t > 0:
                hprev = HT[:, (t - 1) * BL : t * BL]
                for g in range(4):
                    nc.tensor.matmul(
                        ps_cur[:, s, g * BL : (g + 1) * BL],
                        whh_sb[:, g * K : (g + 1) * K],
                        hprev,
                        start=False, stop=(g == 3 and s == BLK - 1),
                        skip_group_check=True,
                    )
            if s == 0 and t > 0:
                b = t // BLK
                if b + 1 < nblk:
                    ps_next = issue_xmm(b + 1)
                pend_head = (issue_heads(b - 1), b - 1)

            # Gate order i,f,g,o: one sigmoid covers i,f and the
            # 2x-prescaled g as soon as the g-matmul lands; the o-matmul
            # and sigma(o) run off the critical path.
            sg = sgp.tile([K, 4 * BL], BF16)
            nc.scalar.activation(sg[:, 0 : 3 * BL], ps_cur[:, s, 0 : 3 * BL],
                                 AF.Sigmoid)

            # c = f*c_prev + i*(2*sg_g - 1)
            A = tmpp.tile([K, BL], BF16, name="A")
            nc.vector.tensor_mul(A[:], sg[:, 0:BL], sg[:, 2 * BL : 3 * BL])
            if t == 0:
                cnew = cp.tile([K, BL], F32, name="cnew")
                nc.vector.scalar_tensor_tensor(
                    cnew[:], A[:], 2.0, sg[:, 0:BL], ALU.mult, ALU.subtract
                )
            else:
                m1 = tmpp.tile([K, BL], F32, name="m1")
                nc.vector.tensor_mul(m1[:], sg[:, BL : 2 * BL], cprev[:])
                Bt = tmpp.tile([K, BL], BF16, name="Bt")
                nc.vector.scalar_tensor_tensor(
                    Bt[:], A[:], 2.0, sg[:, 0:BL], ALU.mult, ALU.subtract
                )
                cnew = cp.tile([K, BL], F32, name="cnew")
                nc.vector.tensor_add(cnew[:], m1[:], Bt[:])
            nc.scalar.activation(sg[:, 3 * BL : 4 * BL],
                                 ps_cur[:, s, 3 * BL : 4 * BL], AF.Sigmoid)
            th = thp.tile([K, BL], F32)
            nc.scalar.activation(th[:], cnew[:], AF.Tanh)
            nc.vector.tensor_mul(
                HT[:, t * BL : (t + 1) * BL], sg[:, 3 * BL : 4 * BL], th[:]
            )
            cprev = cnew

            if pend_head is not None:
                hps, hb = pend_head
                pend_head = None
                ot = outp.tile([K, 2 * OBS], F32)
                nc.vector.tensor_copy(ot[:], hps[:])
                nc.sync.dma_start(heads[hb * K : (hb + 1) * K, :], ot[:])

        # last block's heads
        hps = issue_heads(nblk - 1)
        ot = outp.tile([K, 2 * OBS], F32, name="ot_last")
        nc.vector.tensor_copy(ot[:], hps[:])
        nc.sync.dma_start(heads[(nblk - 1) * K : nblk * K, :], ot[:])
    nc.compile()
    return nc


def _prep_weights(W_ih, W_hh, b_ih, b_hh, W_mu, W_sig):
    # torch gate order i(0:K) f(K:2K) g(2K:3K) o(3K:4K) is kept as-is;
    # the g block is pre-scaled by 2 for tanh(z) = 2*sigmoid(2z) - 1.
    whh_t = np.ascontiguousarray(W_hh.T, np.float32)                # [K, 4K]
    bias = (b_ih + b_hh).astype(np.float32)
    wih_t = np.concatenate(
        [W_ih.T, bias[None, :]], axis=0
    ).astype(np.float32)                                            # [IN+1, 4K]
    whh_t[:, 2 * K : 3 * K] *= 2.0
    wih_t[:, 2 * K : 3 * K] *= 2.0
    wheads = np.concatenate([W_mu.T, W_sig.T], axis=1).astype(np.float32)
    bf = ml_dtypes.bfloat16
    return whh_t.astype(bf), wih_t.astype(bf), wheads.astype(bf)


def kernel(external_input_seq, W_ih, W_hh, b_ih, b_hh, W_mu, b_mu, W_sig, b_sig):
    nsteps = _LSTEPS
    x = np.asarray(external_input_seq, np.float32)[:nsteps]
    W_ih = np.asarray(W_ih, np.float32)
    W_hh = np.asarray(W_hh, np.float32)
    b_ih = np.asarray(b_ih, np.float32)
    b_hh = np.asarray(b_hh, np.float32)
    W_mu = np.asarray(W_mu, np.float32)
    b_mu = np.asarray(b_mu, np.float32)
    W_sig = np.asarray(W_sig, np.float32)
    b_sig = np.asarray(b_sig, np.float32)

    whh_t, wih_t, wheads = _prep_weights(W_ih, W_hh, b_ih, b_hh, W_mu, W_sig)

    if nsteps not in _cache:
        _cache[nsteps] = build_nc(nsteps)
    nc = _cache[nsteps]

    bf = ml_dtypes.bfloat16
    in_maps = []
    for c in range(NCORES):
        xc = x[:, c * BL : (c + 1) * BL, :]              # [nsteps, BL, IN]
        xt = np.empty((IN + 1, nsteps * BL), np.float32)
        xt[:IN] = xc.transpose(2, 0, 1).reshape(IN, nsteps * BL)
        xt[IN] = 1.0
        in_maps.append(
            {"xt": xt.astype(bf), "whh_t": whh_t, "wih_t": wih_t,
             "wheads": wheads}
        )

    res = run_bass_kernel_spmd(
        nc, in_maps, core_ids=list(range(NCORES)), **RUN_KW
    )
    global LAST_RESULT
    LAST_RESULT = res

    mu = np.empty((nsteps, B, OBS), np.float32)
    sig = np.empty((nsteps, B, OBS), np.float32)
    for c in range(NCORES):
        h = res.results[c]["heads"].reshape(nsteps, BL, 2 * OBS)
        mu[:, c * BL : (c + 1) * BL, :] = h[:, :, :OBS]
        sig[:, c * BL : (c + 1) * BL, :] = h[:, :, OBS:]
    mu += b_mu
    sig += b_sig
    return mu, sig


# revision 23
# speedup vs baseline: 2.4976x; 1.0082x over previous
"""DeepAR LSTM kernel for 8 Trainium2 NeuronCores.

Strategy (data-parallel over batch, 256 -> 8 cores x 32):
  * Transposed state layout: h/c are [K=128 partitions, B_loc=32 free].
  * bf16 matmul inputs everywhere (fp32 PSUM accumulation): single-pass
    matmuls + fast weight load, vs fp32's HIGH/LOW double-pass.
  * x-side gate contributions (Wx@x + bias) are precomputed per 4-step
    block directly INTO the PSUM bank (start=True), off the critical
    path; the per-step recurrent matmuls accumulate on top (start=False).
    Bias is folded in via a constant-1 row on x (contraction 65).
  * Gate order i,f,o,g with the g-gate pre-activation scaled by 2 on the
    host so ONE sigmoid instruction covers all four gates:
    tanh(z) = 2*sigmoid(2z) - 1, fixed up in the DVE cell update.
  * mu/logsigma head matmuls run interleaved (one per 4-step block, one
    block behind) on otherwise-idle PE cycles; results stream out via
    DMA during the recurrence. Head biases are added on the host.
"""

import os
import sys
from contextlib import ExitStack

import ml_dtypes
import numpy as np

sys.path.insert(0, "/opt/trn_rl_repo")

import concourse.bass as bass
import concourse.tile as tile
from concourse import bacc, mybir
from concourse.bass_utils import run_bass_kernel_spmd

L, B, IN, K, OBS = 1024, 256, 64, 128, 32
NCORES = 8
BL = B // NCORES  # 32 batch rows per core
CH = 128          # x-chunk length in steps (DMA double-buffered)
BLK = 4           # steps per PSUM gate block (4*128 fp32 = 1 bank)

_LSTEPS = int(os.environ.get("KERNEL_LSTEPS", L))  # smoke-test override

F32 = mybir.dt.float32
F32R = mybir.dt.float32r
BF16 = mybir.dt.bfloat16
AF = mybir.ActivationFunctionType
ALU = mybir.AluOpType

_cache = {}
RUN_KW = {}         # test harness may inject trace=True/tmpdir
LAST_RESULT = None  # BassKernelResults of the most recent run


def build_nc(nsteps: int) -> bass.Bass:
    nc = bacc.Bacc(
        "TRN2", target_bir_lowering=False, debug=False, num_devices=NCORES
    )
    nch = min(CH, nsteps)
    nblk = nsteps // BLK
    xt = nc.dram_tensor("xt", [IN + 1, nsteps * BL], BF16, kind="ExternalInput")
    whh = nc.dram_tensor("whh_t", [K, 4 * K], BF16, kind="ExternalInput")
    wih = nc.dram_tensor("wih_t", [IN + 1, 4 * K], BF16, kind="ExternalInput")
    whd = nc.dram_tensor("wheads", [K, 2 * OBS], BF16, kind="ExternalInput")
    heads = nc.dram_tensor(
        "heads", [nsteps * BL, 2 * OBS], F32, kind="ExternalOutput"
    )

    with ExitStack() as ctx:
        tc = ctx.enter_context(tile.TileContext(nc))
        singles = ctx.enter_context(tc.tile_pool(name="singles", bufs=1))
        xpool = ctx.enter_context(tc.tile_pool(name="xchunk", bufs=2))
        sgp = ctx.enter_context(tc.tile_pool(name="sg", bufs=4))
        cp = ctx.enter_context(tc.tile_pool(name="c", bufs=4))
        thp = ctx.enter_context(tc.tile_pool(name="th", bufs=4))
        tmpp = ctx.enter_context(tc.tile_pool(name="tmp", bufs=4))
        wpsp = ctx.enter_context(tc.tile_pool(name="wps", bufs=1, space="PSUM"))
        gpsp = ctx.enter_context(tc.tile_pool(name="gps", bufs=3, space="PSUM"))
        hpsp = ctx.enter_context(tc.tile_pool(name="hps", bufs=2, space="PSUM"))
        dpsp = ctx.enter_context(tc.tile_pool(name="dps", bufs=1, space="PSUM"))
        outp = ctx.enter_context(tc.tile_pool(name="outt", bufs=3))

        whh_sb = singles.tile([K, 4 * K], BF16)
        nc.sync.dma_start(whh_sb[:], whh[:])
        wih_sb = singles.tile([IN + 1, 4 * K], BF16)
        nc.sync.dma_start(wih_sb[:], wih[:])
        whd_sb = singles.tile([K, 2 * OBS], BF16)
        nc.sync.dma_start(whd_sb[:], whd[:])
        HT = singles.tile([K, nsteps * BL], BF16)

        # A matmul's LDWEIGHTS can carry only ONE sync wait; make PE
        # observe each DMA semaphore via a throwaway 1x1 matmul so real
        # matmuls never need a DMA wait on top of a compute wait.
        dummy_ps = dpsp.tile([1, 1], F32)
        absorb_state = {"first": True}

        def pe_absorb(tile_ap):
            nc.tensor.matmul(
                dummy_ps[:], tile_ap[0:1, 0:1], tile_ap[0:1, 0:1],
                start=absorb_state["first"], stop=False,
                skip_group_check=True,
            )
            absorb_state["first"] = False

        pe_absorb(whh_sb)
        pe_absorb(wih_sb)
        pe_absorb(whd_sb)

        chunk_tiles = {}

        # Keep-warm matmul: no data deps, fills PE idle windows so the
        # HAM clock gate stays at full rate (cold matmuls are 2x slower).
        warm_ps = wpsp.tile([K, 256], F32)

        def pe_warm():
            nc.tensor.matmul(
                warm_ps[:], whh_sb[:, 0:K], whh_sb[:, 0:256],
                start=True, stop=True, skip_group_check=True,
            )

        def load_chunk(c):
            xt_tile = xpool.tile([IN + 1, nch * BL], BF16, name=f"xt_c{c % 2}")
            nc.sync.dma_start(
                xt_tile[:], xt[:, c * nch * BL : (c + 1) * nch * BL]
            )
            pe_absorb(xt_tile)
            chunk_tiles[c] = xt_tile

        def issue_xmm(b):
            # Precompute x-side gate pre-activations for block b straight
            # into a fresh PSUM bank (start=True clears it).
            t0 = b * BLK
            c = t0 // nch
            xt_tile = chunk_tiles[c]
            col0 = (t0 - c * nch) * BL
            ps = gpsp.tile([K, BLK, 4 * BL], F32, name="gateps")
            for g in range(4):
                # start=True pending-zeroes the whole 2KB bank, so only
                # the first matmul of the block may carry it.
                nc.tensor.matmul(
                    ps[:, :, g * BL : (g + 1) * BL],
                    wih_sb[:, g * K : (g + 1) * K],
                    xt_tile[:, col0 : col0 + BLK * BL],
                    start=(g == 0), stop=False, skip_group_check=True,
                )
            return ps

        def issue_heads(b):
            # mu/logsigma heads for (finished) block b: HT chunk is the
            # stationary operand against [W_mu.T | W_sig.T].
            hps = hpsp.tile([K, 2 * OBS], F32, name="headps")
            nc.tensor.matmul(
                hps[:], HT[:, b * K : (b + 1) * K], whd_sb[:],
                start=True, stop=True, skip_group_check=True,
            )
            return hps

        load_chunk(0)
        ps_cur = issue_xmm(0)
        ps_next = issue_xmm(1) if nblk > 1 else None

        cprev = None
        pend_head = None  # (psum_tile, block) awaiting copy+DMA
        for t in range(nsteps):
            s = t % BLK
            if t % BLK == 0 and t > 0:
                ps_cur, ps_next = ps_next, None
            if t % nch == 0 and (t + nch) < nsteps:
                load_chunk(t // nch + 1)

            if t > 0:
                hprev = HT[:, (t - 1) * BL : t * BL]
                for g in range(4):
                    nc.tensor.matmul(
                        ps_cur[:, s, g * BL : (g + 1) * BL],
                        whh_sb[:, g * K : (g + 1) * K],
                        hprev,
                        start=False, stop=(g == 3 and s == BLK - 1),
                        skip_group_check=True,
                    )
            if s == 0 and t > 0:
                b = t // BLK
                if b + 1 < nblk:
                    ps_next = issue_xmm(b + 1)
                pend_head = (issue_heads(b - 1), b - 1)

            # Gate order i,f,g,o: one sigmoid covers i,f and the
            # 2x-prescaled g as soon as the g-matmul lands; the o-matmul
            # and sigma(o) run off the critical path.
            sg = sgp.tile([K, 4 * BL], BF16)
            nc.scalar.activation(sg[:, 0 : 3 * BL], ps_cur[:, s, 0 : 3 * BL],
                                 AF.Sigmoid)

            # c = f*c_prev + i*(2*sg_g - 1)
            A = tmpp.tile([K, BL], BF16, name="A")
            nc.vector.tensor_mul(A[:], sg[:, 0:BL], sg[:, 2 * BL : 3 * BL])
            if t == 0:
                cnew = cp.tile([K, BL], F32, name="cnew")
                nc.vector.scalar_tensor_tensor(
                    cnew[:], A[:], 2.0, sg[:, 0:BL], ALU.mult, ALU.subtract
                )
            else:
                m1 = tmpp.tile([K, BL], F32, name="m1")
                nc.vector.tensor_mul(m1[:], sg[:, BL : 2 * BL], cprev[:])
                Bt = tmpp.tile([K, BL], BF16, name="Bt")
                nc.vector.scalar_tensor_tensor(
                    Bt[:], A[:], 2.0, sg[:, 0:BL], ALU.mult, ALU.subtract
                )
                cnew = cp.tile([K, BL], F32, name="cnew")
                nc.vector.tensor_add(cnew[:], m1[:], Bt[:])
            nc.scalar.activation(sg[:, 3 * BL : 4 * BL],
                                 ps_cur[:, s, 3 * BL : 4 * BL], AF.Sigmoid)
            th = thp.tile([K, BL], BF16)
            nc.scalar.activation(th[:], cnew[:], AF.Tanh)
            nc.vector.tensor_mul(
                HT[:, t * BL : (t + 1) * BL], sg[:, 3 * BL : 4 * BL], th[:]
            )
            cprev = cnew
            if s != 0:
                for _ in range(4):
                    pe_warm()
            else:
                pe_warm()

            if pend_head is not None:
                hps, hb = pend_head
                pend_head = None
                ot = outp.tile([K, 2 * OBS], F32)
                nc.vector.tensor_copy(ot[:], hps[:])
                nc.sync.dma_start(heads[hb * K : (hb + 1) * K, :], ot[:])

        # last block's heads
        hps = issue_heads(nblk - 1)
        ot = outp.tile([K, 2 * OBS], F32, name="ot_last")
        nc.vector.tensor_copy(ot[:], hps[:])
        nc.sync.dma_start(heads[(nblk - 1) * K : nblk * K, :], ot[:])
    nc.compile()
    return nc


def _prep_weights(W_ih, W_hh, b_ih, b_hh, W_mu, W_sig):
    # torch gate order i(0:K) f(K:2K) g(2K:3K) o(3K:4K) is kept as-is;
    # the g block is pre-scaled by 2 for tanh(z) = 2*sigmoid(2z) - 1.
    whh_t = np.ascontiguousarray(W_hh.T, np.float32)                # [K, 4K]
    bias = (b_ih + b_hh).astype(np.float32)
    wih_t = np.concatenate(
        [W_ih.T, bias[None, :]], axis=0
    ).astype(np.float32)                                            # [IN+1, 4K]
    whh_t[:, 2 * K : 3 * K] *= 2.0
    wih_t[:, 2 * K : 3 * K] *= 2.0
    wheads = np.concatenate([W_mu.T, W_sig.T], axis=1).astype(np.float32)
    bf = ml_dtypes.bfloat16
    return whh_t.astype(bf), wih_t.astype(bf), wheads.astype(bf)


def kernel(external_input_seq, W_ih, W_hh, b_ih, b_hh, W_mu, b_mu, W_sig, b_sig):
    nsteps = _LSTEPS
    x = np.asarray(external_input_seq, np.float32)[:nsteps]
    W_ih = np.asarray(W_ih, np.float32)
    W_hh = np.asarray(W_hh, np.float32)
    b_ih = np.asarray(b_ih, np.float32)
    b_hh = np.asarray(b_hh, np.float32)
    W_mu = np.asarray(W_mu, np.float32)
    b_mu = np.asarray(b_mu, np.float32)
    W_sig = np.asarray(W_sig, np.float32)
    b_sig = np.asarray(b_sig, np.float32)

    whh_t, wih_t, wheads = _prep_weights(W_ih, W_hh, b_ih, b_hh, W_mu, W_sig)

    if nsteps not in _cache:
        _cache[nsteps] = build_nc(nsteps)
    nc = _cache[nsteps]

    bf = ml_dtypes.bfloat16
    in_maps = []
    for c in range(NCORES):
        xc = x[:, c * BL : (c + 1) * BL, :]              # [nsteps, BL, IN]
        xt = np.empty((IN + 1, nsteps * BL), np.float32)
        xt[:IN] = xc.transpose(2, 0, 1).reshape(IN, nsteps * BL)
        xt[IN] = 1.0
        in_maps.append(
            {"xt": xt.astype(bf), "whh_t": whh_t, "wih_t": wih_t, "wheads": wheads}
        )

    res = run_bass_kernel_spmd(
        nc, in_maps, core_ids=list(range(NCORES)), **RUN_KW
    )
    global LAST_RESULT
    LAST_RESULT = res

    mu = np.empty((nsteps, B, OBS), np.float32)
    sig = np.empty((nsteps, B, OBS), np.float32)
    for c in range(NCORES):
        h = res.results[c]["heads"].reshape(nsteps, BL, 2 * OBS)
        mu[:, c * BL : (c + 1) * BL, :] = h[:, :, :OBS]
        sig[:, c * BL : (c + 1) * BL, :] = h[:, :, OBS:]
    mu += b_mu
    sig += b_sig
    return mu, sig
